# revision 34
# baseline (speedup 1.0000x reference)
"""Cross-attention kernel for Trainium2 (Bass/Tile), 8 NeuronCores.

Computes, per batch b:
    S   = (dom @ ref^T) * SCALE          [N, N]
    P   = softmax(S, axis=-1)
    x   = P @ ref                        [N, C]
    y   = scramble(x)  (x.T flattened and re-chunked into N rows of C)
    out = y @ proj_w^T + proj_b

The scramble + linear fuse algebraically:
    out[2*cp + e, j] = sum_q x[512*e + q, cp] * proj_w[j, q] + proj_b[j]
so out rows with parity e are (x_half_e^T @ proj_w^T) — computed on the
tensor engine with x tiles as lhsT directly (natural layout, no transpose)
and a host-pretransposed proj_w^T as the moving operand; the row interleave
(2*cp + e) is folded into the output DMA access pattern.

Sharding: data-parallel over batch. B=16 -> 2 batches per core, 8 cores,
no collectives.

All matmul operands are bf16 (host-cast): same 1 cyc/row streaming as
fp32r but PE transposes drop to 1.0 cyc/row, LDWEIGHTS halves, and input
HBM traffic halves (measured end-to-end rel err ~6e-3 vs the 2e-2 gate).
PSUM accumulation stays fp32.

Emission is software-pipelined across the 16 (batch, query-tile) units:
QK(k) -> x(k-1) -> transposes(k), so the PE fills the exp-eviction and
Pt-copy latencies of unit k with the x matmuls of unit k-1 instead of
stalling (the PE also downclocks when idle, doubling every bubble).

DMA rings (measured: SWDGE ~200GB/s, each HWDGE ~70GB/s, flow start
jitters by ~2us per ring): the three tensors gating the first QK unit
(domT0.h0, refT0.h0, refT0.h1) ride one ring EACH so no single ring's
startup lag serializes them; natural ref streams next on SWDGE for
x(0); batch-1 inputs follow; out stores round-robin with the final
tiles on SWDGE. One merged dispatch per tensor-half (a ~700ns engine
dispatch cost makes many small dma_starts expensive). A burst of
dependency-free matmuls on zeros at t=0 ramps the PE clock (HAM) while
the first loads stream, ending as the gating loads land; a few filler
matmuls before the first h1 group absorb its arrival jitter.
"""

import os
from contextlib import ExitStack

import numpy as np
import ml_dtypes

import concourse.bass as bass
import concourse.mybir as mybir
import concourse.tile as tile
from concourse import bacc
from concourse._compat import with_exitstack
from concourse.bass_utils import run_bass_kernel_spmd

B, N, C = 16, 1024, 512
NUM_HEADS = 8
SCALE = (C // NUM_HEADS) ** -0.5  # 0.125
CORES = 8
BPC = B // CORES  # batches per core

P = 128          # partitions
NT = N // P      # 8 query tiles
CCH = C // P     # 4 contraction chunks over channels
MH = N // 512    # 2 key halves (PSUM bank = 512 fp32)
MCH = N // P     # 8 key chunks
JT = C // P      # 4 output-column blocks per half

F32 = mybir.dt.float32
BF16 = mybir.dt.bfloat16

WARMUP_MMS = int(os.environ.get("KERNEL_WARMUP", "16"))
FILL_H = int(os.environ.get("KERNEL_FILL_H", "6"))


@with_exitstack
def _core_kernel(ctx: ExitStack, tc: tile.TileContext,
                 domt_d, reft_d, ref_d, wt_d, bias_d, eye_d, out_d):
    nc = tc.nc

    consts = ctx.enter_context(tc.tile_pool(name="consts", bufs=1))
    identity = consts.tile([P, P], BF16)

    ps_S = ctx.enter_context(tc.tile_pool(name="ps_s", bufs=3, space="PSUM"))
    ps_T = ctx.enter_context(tc.tile_pool(name="ps_t", bufs=2, space="PSUM"))
    ps_X = ctx.enter_context(tc.tile_pool(name="ps_x", bufs=3, space="PSUM"))

    # PE warmup: dense dependency-free matmuls on memset zeros while the
    # first input DMAs stream, so the HAM clock gate reaches 8/8 before
    # real work. Sized to end as the first QK operands land. The same
    # tile also backs `filler()` matmuls that keep the PE busy (and
    # clocked) across the few unavoidable early DMA waits.
    zr = consts.tile([P, 640], BF16)
    nc.vector.memset(zr[:], 0.0)
    warm_ps = ps_T.tile([P, 512], F32, tag="ps_t")

    def filler(n):
        for _ in range(n):
            nc.tensor.matmul(warm_ps[:], zr[:, :P], zr[:, P:640],
                             start=True, stop=True)

    filler(WARMUP_MMS)

    p_ref = ctx.enter_context(tc.tile_pool(name="ref", bufs=2))
    p_domT = ctx.enter_context(tc.tile_pool(name="domT", bufs=2))
    p_refT = ctx.enter_context(tc.tile_pool(name="refT", bufs=2))
    p_P = ctx.enter_context(tc.tile_pool(name="probs", bufs=2))
    p_Pt = ctx.enter_context(tc.tile_pool(name="probsT", bufs=2))
    p_x = ctx.enter_context(tc.tile_pool(name="x", bufs=8))
    p_out = ctx.enter_context(tc.tile_pool(name="out", bufs=4))
    p_stats = ctx.enter_context(tc.tile_pool(name="stats", bufs=8))

    # ---- pre-emit every input DMA so the rings stream continuously ----
    def load_T_half(sb, dr, b, h, eng, ksplit=1):
        # sb free layout: chunk k at cols [k*N, (k+1)*N); h-split so the
        # first half's matmuls unlock after half the bytes land. ksplit
        # further chops the k (contraction) chunks so the first QK
        # matmuls of a group unlock per-chunk as bytes stream in.
        v = sb[:].rearrange("p (k n) -> p k n", k=CCH)
        s = dr[b].rearrange("(k p) n -> p k n", p=P)
        kn = CCH // ksplit
        for i in range(ksplit):
            eng.dma_start(
                v[:, i * kn:(i + 1) * kn, h * 512:(h + 1) * 512],
                s[:, i * kn:(i + 1) * kn, h * 512:(h + 1) * 512],
            )

    def load_nat(sb, dr, b, t0, nt_, eng):
        # tile col block t holds rows [128t, 128(t+1)) of the [N, C] matrix
        eng.dma_start(
            sb[:, t0 * C:(t0 + nt_) * C].rearrange("p (t c) -> p t c", t=nt_),
            dr[b, t0 * P:(t0 + nt_) * P].rearrange("(t p) c -> p t c", p=P),
        )

    domT_sbs = [p_domT.tile([P, CCH * N], BF16, tag="domT", name=f"domT_sb{i}")
                for i in range(BPC)]
    refT_sbs = [p_refT.tile([P, CCH * N], BF16, tag="refT", name=f"refT_sb{i}")
                for i in range(BPC)]
    ref_sbs = [p_ref.tile([P, NT * C], BF16, tag="ref", name=f"ref_sb{i}")
               for i in range(BPC)]
    wt_sb = consts.tile([P, CCH * C], BF16)
    bias_sb = consts.tile([P, C], F32)

    def load_T_cols(sb, dr, b, c0, c1, eng):
        # query-column sub-range of a [C, N]-transposed tensor: unit nt
        # reads cols [nt*P, (nt+1)*P) of every k chunk, so QK(0) unlocks
        # on a 128KB slice instead of the full 512KB half
        v = sb[:].rearrange("p (k n) -> p k n", k=CCH)
        s = dr[b].rearrange("(k p) n -> p k n", p=P)
        eng.dma_start(v[:, :, c0:c1], s[:, :, c0:c1])

    # SWDGE: first-QK lhsT in query-split slices that stream just ahead
    # of the per-unit pipeline, then natural ref for x(0), then the rest
    load_T_cols(domT_sbs[0], domt_d, 0, 0, P, nc.gpsimd)        # nt0
    load_T_cols(domT_sbs[0], domt_d, 0, P, 2 * P, nc.gpsimd)    # nt1
    load_T_cols(domT_sbs[0], domt_d, 0, 2 * P, 4 * P, nc.gpsimd)  # nt2-3
    load_nat(ref_sbs[0], ref_d, 0, 0, 4, nc.gpsimd)
    load_T_half(domT_sbs[0], domt_d, 0, 1, nc.gpsimd)
    nc.gpsimd.dma_start(
        wt_sb[:].rearrange("p (k c) -> p k c", k=CCH),
        wt_d.rearrange("(k p) c -> p k c", p=P),
    )
    load_T_half(domT_sbs[1], domt_d, 1, 0, nc.gpsimd)
    load_T_half(domT_sbs[1], domt_d, 1, 1, nc.gpsimd)
    # sync HWDGE: kept input-only until ~mid-kernel (stores would queue
    # ahead of batch-1 loads and stall the out-tile pool otherwise)
    nc.sync.dma_start(identity[:], eye_d[:, :])
    load_T_half(refT_sbs[0], reft_d, 0, 0, nc.sync)
    load_nat(ref_sbs[0], ref_d, 0, 4, 4, nc.sync)
    nc.sync.dma_start(bias_sb[:], bias_d.partition_broadcast(P))
    load_T_half(refT_sbs[1], reft_d, 1, 0, nc.sync)
    load_nat(ref_sbs[1], ref_d, 1, 0, 4, nc.sync)
    # scalar HWDGE
    load_T_half(refT_sbs[0], reft_d, 0, 1, nc.scalar)
    load_T_half(refT_sbs[1], reft_d, 1, 1, nc.scalar)
    load_nat(ref_sbs[1], ref_d, 1, 4, 4, nc.scalar)

    # per-(batch, parity) store rings: sync only once its inputs drained;
    # the final half ends on the fast SWDGE ring
    store_engs = {
        (0, 0): [nc.gpsimd, nc.scalar, nc.gpsimd, nc.scalar],
        (0, 1): [nc.scalar, nc.gpsimd, nc.scalar, nc.gpsimd],
        (1, 0): [nc.gpsimd, nc.sync, nc.scalar, nc.gpsimd],
        (1, 1): [nc.sync, nc.scalar, nc.gpsimd, nc.gpsimd],
    }

    x_tiles = {0: [], 1: []}
    P_sbs = {}
    Pt_tiles = {}
    recips = {}

    def emit_half_out(b, e):
        # out rows (2*cp + e) = x_half_e^T @ proj_w^T + bias
        out_v = out_d[b].rearrange("(n2 two) j -> two n2 j", two=2)
        for cb in range(JT):
            ps_z = ps_X.tile([P, C], F32, tag="ps_x")
            for q in range(CCH):
                x_t = x_tiles[b][e * CCH + q]  # q-chunk of half e
                nc.tensor.matmul(
                    ps_z[:],
                    x_t[:, cb * P:(cb + 1) * P],
                    wt_sb[:, q * C:(q + 1) * C],
                    start=(q == 0),
                    stop=(q == CCH - 1),
                )
            o_sb = p_out.tile([P, C], F32, tag="out")
            nc.vector.tensor_add(o_sb[:], ps_z[:], bias_sb[:])
            store_engs[(b, e)][cb].dma_start(
                out_v[e, cb * P:(cb + 1) * P, :], o_sb[:])

    def stage1a(k):
        # ---- S = dom @ ref^T; P = exp(S*SCALE) per half, fused rowsums
        # logits are bounded (~|16|) so the max-subtraction is unnecessary
        b, nt = divmod(k, NT)
        domT_sb, refT_sb = domT_sbs[b], refT_sbs[b]
        P_sb = p_P.tile([P, N], BF16, tag="probs", name=f"P_sb{k}")
        rowsums = []
        for h in range(MH):
            if k == 0 and h == 1:
                filler(FILL_H)  # cover refT0.h1's in-flight tail
            ps_s = ps_S.tile([P, 512], F32, tag="ps_s", name=f"ps_s{k}_{h}")
            for kk in range(CCH):
                nc.tensor.matmul(
                    ps_s[:],
                    domT_sb[:, kk * N + nt * P: kk * N + (nt + 1) * P],
                    refT_sb[:, kk * N + h * 512: kk * N + (h + 1) * 512],
                    start=(kk == 0), stop=(kk == CCH - 1),
                )
            rs = p_stats.tile([P, 1], F32, tag="rowsum", name=f"rs{k}_{h}")
            nc.scalar.activation(P_sb[:, h * 512:(h + 1) * 512], ps_s[:],
                                 mybir.ActivationFunctionType.Exp,
                                 scale=float(SCALE), accum_out=rs[:])
            rowsums.append(rs)
        rowsum = p_stats.tile([P, 1], F32, tag="rowsum2", name=f"rsum{k}")
        nc.vector.tensor_add(rowsum[:], rowsums[0][:], rowsums[1][:])
        recip = p_stats.tile([P, 1], F32, tag="recip", name=f"recip{k}")
        nc.vector.reciprocal(recip[:], rowsum[:])
        recips[k] = recip
        P_sbs[k] = P_sb

    def stage1b(k):
        # ---- transpose P -> Pt (chunk mi at cols [mi*P, (mi+1)*P)) ----
        # bf16: a [128, 1024] transpose psum fits one bank, so all 8
        # blocks go into a single accumulation group
        P_sb = P_sbs.pop(k)
        Pt_sb = p_Pt.tile([P, N], BF16, tag="probsT", name=f"Pt_sb{k}")
        ps = ps_T.tile([P, N], BF16, tag="ps_t", name=f"ps_t{k}")
        for mi in range(MCH):
            nc.tensor.transpose(ps[:, mi * P:(mi + 1) * P],
                                P_sb[:, mi * P:(mi + 1) * P],
                                identity[:])
        nc.scalar.copy(Pt_sb[:, :512], ps[:, :512])
        nc.vector.tensor_copy(Pt_sb[:, 512:], ps[:, 512:])
        Pt_tiles[k] = Pt_sb

    def stage2(k):
        # ---- x = P @ ref ----
        b, nt = divmod(k, NT)
        Pt_sb = Pt_tiles.pop(k)
        ps_x = ps_X.tile([P, C], F32, tag="ps_x", name=f"ps_x{k}")
        for mi in range(MCH):
            nc.tensor.matmul(
                ps_x[:],
                Pt_sb[:, mi * P:(mi + 1) * P],
                ref_sbs[b][:, mi * C:(mi + 1) * C],
                start=(mi == 0), stop=(mi == MCH - 1),
            )
        # evict with fused softmax normalization (per-row 1/sum)
        x_t = p_x.tile([P, C], BF16, tag="x", name=f"x_t{k}")
        nc.vector.tensor_scalar_mul(x_t[:], ps_x[:], recips.pop(k)[:])
        x_tiles[b].append(x_t)

        # projection for a half as soon as its 4 x tiles exist
        if nt == CCH - 1:
            emit_half_out(b, 0)
        elif nt == NT - 1:
            emit_half_out(b, 1)

    for k in range(BPC * NT):
        stage1a(k)
        if k > 0:
            stage2(k - 1)
        stage1b(k)
    stage2(BPC * NT - 1)


_CACHED = {}


def _build():
    key = ("nc", WARMUP_MMS, FILL_H)
    if key in _CACHED:
        return _CACHED[key]
    nc = bacc.Bacc("TRN2", target_bir_lowering=False, debug=False)
    domt_d = nc.dram_tensor("domt", [BPC, C, N], BF16, kind="ExternalInput").ap()
    reft_d = nc.dram_tensor("reft", [BPC, C, N], BF16, kind="ExternalInput").ap()
    ref_d = nc.dram_tensor("ref", [BPC, N, C], BF16, kind="ExternalInput").ap()
    wt_d = nc.dram_tensor("wt", [C, C], BF16, kind="ExternalInput").ap()
    bias_d = nc.dram_tensor("bias", [C], F32, kind="ExternalInput").ap()
    eye_d = nc.dram_tensor("eye", [P, P], BF16, kind="ExternalInput").ap()
    out_d = nc.dram_tensor("out", [BPC, N, C], F32, kind="ExternalOutput").ap()

    with tile.TileContext(nc) as tc:
        _core_kernel(tc, domt_d, reft_d, ref_d, wt_d, bias_d, eye_d, out_d)
    nc.compile()
    _CACHED[key] = nc
    return nc


LAST_RESULTS = None


def kernel(dom, ref, proj_w, proj_b):
    global LAST_RESULTS
    bf = ml_dtypes.bfloat16
    dom = np.asarray(dom, dtype=np.float32)
    ref = np.asarray(ref, dtype=np.float32)
    wt = np.ascontiguousarray(np.asarray(proj_w, dtype=np.float32).T.astype(bf))
    bias = np.ascontiguousarray(np.asarray(proj_b, dtype=np.float32))
    eye = np.eye(P, dtype=bf)

    domt = np.ascontiguousarray(dom.transpose(0, 2, 1).astype(bf))
    reft = np.ascontiguousarray(ref.transpose(0, 2, 1).astype(bf))
    refb = np.ascontiguousarray(ref.astype(bf))
    nc = _build()
    in_maps = [
        {
            "domt": domt[c * BPC:(c + 1) * BPC],
            "reft": reft[c * BPC:(c + 1) * BPC],
            "ref": refb[c * BPC:(c + 1) * BPC],
            "wt": wt,
            "bias": bias,
            "eye": eye,
        }
        for c in range(CORES)
    ]
    res = run_bass_kernel_spmd(nc, in_maps, list(range(CORES)))
    LAST_RESULTS = res
    if res.exec_time_ns is not None:
        print(f"HW exec time: {res.exec_time_ns} ns")
    return np.concatenate([r["out"] for r in res.results], axis=0)


# revision 35
# speedup vs baseline: 1.1680x; 1.1680x over previous
"""Cross-attention kernel for Trainium2 (Bass/Tile), 8 NeuronCores.

Computes, per batch b:
    S   = (dom @ ref^T) * SCALE          [N, N]
    P   = softmax(S, axis=-1)
    x   = P @ ref                        [N, C]
    y   = scramble(x)  (x.T flattened and re-chunked into N rows of C)
    out = y @ proj_w^T + proj_b

The scramble + linear fuse algebraically:
    out[2*cp + e, j] = sum_q x[512*e + q, cp] * proj_w[j, q] + proj_b[j]
so out rows with parity e are (x_half_e^T @ proj_w^T) — computed on the
tensor engine with x tiles as lhsT directly (natural layout, no transpose)
and a host-pretransposed proj_w^T as the moving operand; the row interleave
(2*cp + e) is folded into the output DMA access pattern.

Sharding: data-parallel over batch. B=16 -> 2 batches per core, 8 cores,
no collectives.

All matmul operands are bf16 (host-cast): same 1 cyc/row streaming as
fp32r but PE transposes drop to 1.0 cyc/row, LDWEIGHTS halves, and input
HBM traffic halves (measured end-to-end rel err ~6e-3 vs the 2e-2 gate).
PSUM accumulation stays fp32.

Emission is software-pipelined across the 16 (batch, query-tile) units:
QK(k) -> x(k-1) -> transposes(k), so the PE fills the exp-eviction and
Pt-copy latencies of unit k with the x matmuls of unit k-1 instead of
stalling (the PE also downclocks when idle, doubling every bubble).

DMA rings (measured: SWDGE ~200GB/s, each HWDGE ~70GB/s, flow start
jitters by ~2us per ring): the three tensors gating the first QK unit
(domT0.h0, refT0.h0, refT0.h1) ride one ring EACH so no single ring's
startup lag serializes them; natural ref streams next on SWDGE for
x(0); batch-1 inputs follow; out stores round-robin with the final
tiles on SWDGE. One merged dispatch per tensor-half (a ~700ns engine
dispatch cost makes many small dma_starts expensive). A burst of
dependency-free matmuls on zeros at t=0 ramps the PE clock (HAM) while
the first loads stream, ending as the gating loads land; a few filler
matmuls before the first h1 group absorb its arrival jitter.
"""

import os
from contextlib import ExitStack

import numpy as np
import ml_dtypes

import concourse.bass as bass
import concourse.mybir as mybir
import concourse.tile as tile
from concourse import bacc
from concourse._compat import with_exitstack
from concourse.bass_utils import run_bass_kernel_spmd

B, N, C = 16, 1024, 512
NUM_HEADS = 8
SCALE = (C // NUM_HEADS) ** -0.5  # 0.125
CORES = 8
BPC = B // CORES  # batches per core

P = 128          # partitions
NT = N // P      # 8 query tiles
CCH = C // P     # 4 contraction chunks over channels
MH = N // 512    # 2 key halves (PSUM bank = 512 fp32)
MCH = N // P     # 8 key chunks
JT = C // P      # 4 output-column blocks per half

F32 = mybir.dt.float32
BF16 = mybir.dt.bfloat16

WARMUP_MMS = int(os.environ.get("KERNEL_WARMUP", "14"))
FILL_H = int(os.environ.get("KERNEL_FILL_H", "6"))


@with_exitstack
def _core_kernel(ctx: ExitStack, tc: tile.TileContext,
                 domt_d, reft_d, ref_d, wt_d, bias_d, eye_d, out_d):
    nc = tc.nc

    consts = ctx.enter_context(tc.tile_pool(name="consts", bufs=1))
    identity = consts.tile([P, P], BF16)

    ps_S = ctx.enter_context(tc.tile_pool(name="ps_s", bufs=3, space="PSUM"))
    ps_T = ctx.enter_context(tc.tile_pool(name="ps_t", bufs=2, space="PSUM"))
    ps_X = ctx.enter_context(tc.tile_pool(name="ps_x", bufs=3, space="PSUM"))

    # PE warmup: dense dependency-free matmuls on memset zeros while the
    # first input DMAs stream, so the HAM clock gate reaches 8/8 before
    # real work. Sized to end as the first QK operands land. The same
    # tile also backs `filler()` matmuls that keep the PE busy (and
    # clocked) across the few unavoidable early DMA waits.
    zr = consts.tile([P, 640], BF16)
    nc.vector.memset(zr[:], 0.0)
    warm_ps = ps_T.tile([P, 512], F32, tag="ps_t")

    def filler(n):
        for _ in range(n):
            nc.tensor.matmul(warm_ps[:], zr[:, :P], zr[:, P:640],
                             start=True, stop=True)

    filler(WARMUP_MMS)

    p_ref = ctx.enter_context(tc.tile_pool(name="ref", bufs=2))
    p_domT = ctx.enter_context(tc.tile_pool(name="domT", bufs=2))
    p_refT = ctx.enter_context(tc.tile_pool(name="refT", bufs=2))
    p_P = ctx.enter_context(tc.tile_pool(name="probs", bufs=2))
    p_Pt = ctx.enter_context(tc.tile_pool(name="probsT", bufs=2))
    p_x = ctx.enter_context(tc.tile_pool(name="x", bufs=8))
    p_out = ctx.enter_context(tc.tile_pool(name="out", bufs=4))
    p_stats = ctx.enter_context(tc.tile_pool(name="stats", bufs=8))

    # ---- pre-emit every input DMA so the rings stream continuously ----
    def load_T_half(sb, dr, b, h, eng, ksplit=1):
        # sb free layout: chunk k at cols [k*N, (k+1)*N); h-split so the
        # first half's matmuls unlock after half the bytes land. ksplit
        # further chops the k (contraction) chunks so the first QK
        # matmuls of a group unlock per-chunk as bytes stream in.
        v = sb[:].rearrange("p (k n) -> p k n", k=CCH)
        s = dr[b].rearrange("(k p) n -> p k n", p=P)
        kn = CCH // ksplit
        for i in range(ksplit):
            eng.dma_start(
                v[:, i * kn:(i + 1) * kn, h * 512:(h + 1) * 512],
                s[:, i * kn:(i + 1) * kn, h * 512:(h + 1) * 512],
            )

    def load_nat(sb, dr, b, t0, nt_, eng):
        # tile col block t holds rows [128t, 128(t+1)) of the [N, C] matrix
        eng.dma_start(
            sb[:, t0 * C:(t0 + nt_) * C].rearrange("p (t c) -> p t c", t=nt_),
            dr[b, t0 * P:(t0 + nt_) * P].rearrange("(t p) c -> p t c", p=P),
        )

    domT_sbs = [p_domT.tile([P, CCH * N], BF16, tag="domT", name=f"domT_sb{i}")
                for i in range(BPC)]
    refT_sbs = [p_refT.tile([P, CCH * N], BF16, tag="refT", name=f"refT_sb{i}")
                for i in range(BPC)]
    ref_sbs = [p_ref.tile([P, NT * C], BF16, tag="ref", name=f"ref_sb{i}")
               for i in range(BPC)]
    wt_sb = consts.tile([P, CCH * C], BF16)
    bias_sb = consts.tile([P, C], F32)

    def load_T_cols(sb, dr, b, c0, c1, eng):
        # query-column sub-range of a [C, N]-transposed tensor: unit nt
        # reads cols [nt*P, (nt+1)*P) of every k chunk, so QK(0) unlocks
        # on a 128KB slice instead of the full 512KB half
        v = sb[:].rearrange("p (k n) -> p k n", k=CCH)
        s = dr[b].rearrange("(k p) n -> p k n", p=P)
        eng.dma_start(v[:, :, c0:c1], s[:, :, c0:c1])

    # SWDGE: first-QK lhsT in query-split slices that stream just ahead
    # of the per-unit pipeline, then natural ref for x(0), then the rest
    load_T_cols(domT_sbs[0], domt_d, 0, 0, P, nc.gpsimd)        # nt0
    load_T_cols(domT_sbs[0], domt_d, 0, P, 2 * P, nc.gpsimd)    # nt1
    load_T_cols(domT_sbs[0], domt_d, 0, 2 * P, 4 * P, nc.gpsimd)  # nt2-3
    load_nat(ref_sbs[0], ref_d, 0, 0, 4, nc.gpsimd)
    load_nat(ref_sbs[0], ref_d, 0, 4, 4, nc.gpsimd)
    load_T_half(domT_sbs[0], domt_d, 0, 1, nc.gpsimd)
    nc.gpsimd.dma_start(
        wt_sb[:].rearrange("p (k c) -> p k c", k=CCH),
        wt_d.rearrange("(k p) c -> p k c", p=P),
    )
    load_T_half(domT_sbs[1], domt_d, 1, 0, nc.gpsimd)
    load_T_half(domT_sbs[1], domt_d, 1, 1, nc.gpsimd)
    # sync HWDGE: kept input-only until ~mid-kernel (stores would queue
    # ahead of batch-1 loads and stall the out-tile pool otherwise)
    nc.sync.dma_start(identity[:], eye_d[:, :])
    load_T_half(refT_sbs[0], reft_d, 0, 0, nc.sync)
    nc.sync.dma_start(bias_sb[:], bias_d.partition_broadcast(P))
    load_T_half(refT_sbs[1], reft_d, 1, 0, nc.sync)
    load_nat(ref_sbs[1], ref_d, 1, 0, 4, nc.sync)
    # scalar HWDGE
    load_T_half(refT_sbs[0], reft_d, 0, 1, nc.scalar)
    load_T_half(refT_sbs[1], reft_d, 1, 1, nc.scalar)
    load_nat(ref_sbs[1], ref_d, 1, 4, 4, nc.scalar)

    # per-(batch, parity) store rings: sync only once its inputs drained;
    # the final half ends on the fast SWDGE ring
    store_engs = {
        (0, 0): [nc.gpsimd, nc.scalar, nc.gpsimd, nc.scalar],
        (0, 1): [nc.scalar, nc.gpsimd, nc.scalar, nc.gpsimd],
        (1, 0): [nc.gpsimd, nc.sync, nc.scalar, nc.gpsimd],
        (1, 1): [nc.sync, nc.scalar, nc.gpsimd, nc.gpsimd],
    }

    x_tiles = {0: [], 1: []}
    P_sbs = {}
    Pt_tiles = {}
    recips = {}

    def emit_half_out(b, e):
        # out rows (2*cp + e) = x_half_e^T @ proj_w^T + bias
        out_v = out_d[b].rearrange("(n2 two) j -> two n2 j", two=2)
        for cb in range(JT):
            ps_z = ps_X.tile([P, C], F32, tag="ps_x")
            for q in range(CCH):
                x_t = x_tiles[b][e * CCH + q]  # q-chunk of half e
                nc.tensor.matmul(
                    ps_z[:],
                    x_t[:, cb * P:(cb + 1) * P],
                    wt_sb[:, q * C:(q + 1) * C],
                    start=(q == 0),
                    stop=(q == CCH - 1),
                )
            o_sb = p_out.tile([P, C], F32, tag="out")
            nc.vector.tensor_add(o_sb[:], ps_z[:], bias_sb[:])
            store_engs[(b, e)][cb].dma_start(
                out_v[e, cb * P:(cb + 1) * P, :], o_sb[:])

    def stage1a(k):
        # ---- S = dom @ ref^T; P = exp(S*SCALE) per half, fused rowsums
        # logits are bounded (~|16|) so the max-subtraction is unnecessary
        b, nt = divmod(k, NT)
        domT_sb, refT_sb = domT_sbs[b], refT_sbs[b]
        P_sb = p_P.tile([P, N], BF16, tag="probs", name=f"P_sb{k}")
        rowsums = []
        for h in range(MH):
            if k == 0 and h == 1:
                filler(FILL_H)  # cover refT0.h1's in-flight tail
            ps_s = ps_S.tile([P, 512], F32, tag="ps_s", name=f"ps_s{k}_{h}")
            for kk in range(CCH):
                nc.tensor.matmul(
                    ps_s[:],
                    domT_sb[:, kk * N + nt * P: kk * N + (nt + 1) * P],
                    refT_sb[:, kk * N + h * 512: kk * N + (h + 1) * 512],
                    start=(kk == 0), stop=(kk == CCH - 1),
                )
            rs = p_stats.tile([P, 1], F32, tag="rowsum", name=f"rs{k}_{h}")
            nc.scalar.activation(P_sb[:, h * 512:(h + 1) * 512], ps_s[:],
                                 mybir.ActivationFunctionType.Exp,
                                 scale=float(SCALE), accum_out=rs[:])
            rowsums.append(rs)
        rowsum = p_stats.tile([P, 1], F32, tag="rowsum2", name=f"rsum{k}")
        nc.vector.tensor_add(rowsum[:], rowsums[0][:], rowsums[1][:])
        recip = p_stats.tile([P, 1], F32, tag="recip", name=f"recip{k}")
        nc.vector.reciprocal(recip[:], rowsum[:])
        recips[k] = recip
        P_sbs[k] = P_sb

    def stage1b(k):
        # ---- transpose P -> Pt (chunk mi at cols [mi*P, (mi+1)*P)) ----
        # bf16: a [128, 1024] transpose psum fits one bank, so all 8
        # blocks go into a single accumulation group
        P_sb = P_sbs.pop(k)
        Pt_sb = p_Pt.tile([P, N], BF16, tag="probsT", name=f"Pt_sb{k}")
        ps = ps_T.tile([P, N], BF16, tag="ps_t", name=f"ps_t{k}")
        for mi in range(MCH):
            nc.tensor.transpose(ps[:, mi * P:(mi + 1) * P],
                                P_sb[:, mi * P:(mi + 1) * P],
                                identity[:])
        nc.scalar.copy(Pt_sb[:, :512], ps[:, :512])
        nc.vector.tensor_copy(Pt_sb[:, 512:], ps[:, 512:])
        Pt_tiles[k] = Pt_sb

    def stage2(k):
        # ---- x = P @ ref ----
        b, nt = divmod(k, NT)
        Pt_sb = Pt_tiles.pop(k)
        ps_x = ps_X.tile([P, C], F32, tag="ps_x", name=f"ps_x{k}")
        for mi in range(MCH):
            nc.tensor.matmul(
                ps_x[:],
                Pt_sb[:, mi * P:(mi + 1) * P],
                ref_sbs[b][:, mi * C:(mi + 1) * C],
                start=(mi == 0), stop=(mi == MCH - 1),
            )
        # evict with fused softmax normalization (per-row 1/sum)
        x_t = p_x.tile([P, C], BF16, tag="x", name=f"x_t{k}")
        nc.vector.tensor_scalar_mul(x_t[:], ps_x[:], recips.pop(k)[:])
        x_tiles[b].append(x_t)

        # projection for a half as soon as its 4 x tiles exist
        if nt == CCH - 1:
            emit_half_out(b, 0)
        elif nt == NT - 1:
            emit_half_out(b, 1)

    for k in range(BPC * NT):
        stage1a(k)
        if k > 0:
            stage2(k - 1)
        stage1b(k)
    stage2(BPC * NT - 1)


_CACHED = {}


def _build():
    key = ("nc", WARMUP_MMS, FILL_H)
    if key in _CACHED:
        return _CACHED[key]
    nc = bacc.Bacc("TRN2", target_bir_lowering=False, debug=False)
    domt_d = nc.dram_tensor("domt", [BPC, C, N], BF16, kind="ExternalInput").ap()
    reft_d = nc.dram_tensor("reft", [BPC, C, N], BF16, kind="ExternalInput").ap()
    ref_d = nc.dram_tensor("ref", [BPC, N, C], BF16, kind="ExternalInput").ap()
    wt_d = nc.dram_tensor("wt", [C, C], BF16, kind="ExternalInput").ap()
    bias_d = nc.dram_tensor("bias", [C], F32, kind="ExternalInput").ap()
    eye_d = nc.dram_tensor("eye", [P, P], BF16, kind="ExternalInput").ap()
    out_d = nc.dram_tensor("out", [BPC, N, C], F32, kind="ExternalOutput").ap()

    with tile.TileContext(nc) as tc:
        _core_kernel(tc, domt_d, reft_d, ref_d, wt_d, bias_d, eye_d, out_d)
    nc.compile()
    _CACHED[key] = nc
    return nc


LAST_RESULTS = None


def kernel(dom, ref, proj_w, proj_b):
    global LAST_RESULTS
    bf = ml_dtypes.bfloat16
    dom = np.asarray(dom, dtype=np.float32)
    ref = np.asarray(ref, dtype=np.float32)
    wt = np.ascontiguousarray(np.asarray(proj_w, dtype=np.float32).T.astype(bf))
    bias = np.ascontiguousarray(np.asarray(proj_b, dtype=np.float32))
    eye = np.eye(P, dtype=bf)

    domt = np.ascontiguousarray(dom.transpose(0, 2, 1).astype(bf))
    reft = np.ascontiguousarray(ref.transpose(0, 2, 1).astype(bf))
    refb = np.ascontiguousarray(ref.astype(bf))
    nc = _build()
    in_maps = [
        {
            "domt": domt[c * BPC:(c + 1) * BPC],
            "reft": reft[c * BPC:(c + 1) * BPC],
            "ref": refb[c * BPC:(c + 1) * BPC],
            "wt": wt,
            "bias": bias,
            "eye": eye,
        }
        for c in range(CORES)
    ]
    res = run_bass_kernel_spmd(nc, in_maps, list(range(CORES)))
    LAST_RESULTS = res
    if res.exec_time_ns is not None:
        print(f"HW exec time: {res.exec_time_ns} ns")
    return np.concatenate([r["out"] for r in res.results], axis=0)
